# revision 43
# baseline (speedup 1.0000x reference)
"""Trainium2 Bass kernel for nn_DeepHopfield (self-contained).

8 cores, data-parallel over batch (128 images/core).

Precision split (measured sensitivity): the clustering output is chaotic
(~1e-2 L2 floor under ANY arithmetic change) but only the LABEL branch's
precision moves the error materially (it feeds the Hopfield weight matrix,
which shifts every row's sign decisions coherently). So:
  - label branch: full fp32, sharded 8-way (16 labels/core) + AllGather.
  - image branch: fp32r matmuls (12-bit round-to-nearest mantissa, 2x faster).

Layout notes
  conv1: 4 y-phase replicas [128=(dy4,xi32), (yb8,b)], Toeplitz-x weights,
         M=(xq14,o8), x-pool via even/odd weight split, y-pool via phase pairs.
  conv2: 2 x-phase replicas [128=(xr4,ci32), (xb,18ypad,b)], dy via free offset,
         M=(j2,o64) with dx_eff=dx+j folding, x-pool = j-halves, y-pool free dim.
         B-taps at xp=6 hit zero-padded columns -> matmuls omitted.
  fc1:   image: batch-major accumulate (28 matmuls N=512 fp32r) + 4 transposes.
         label: latent-major (N=16 fp32).
  hopfield: latent-major state; hT = sum_jc w[jc]^T @ s[jc]; energy via
         ones-column matmul; min-select via K=1 broadcast matmul + copy_predicated.
"""
import contextlib

import numpy as np

import concourse.bass as bass
import concourse.bacc as bacc
import concourse.mybir as mybir
import concourse.tile as tile
from concourse import bass_utils

F32 = mybir.dt.float32
F32R = mybir.dt.float32r
AF = mybir.ActivationFunctionType
ALU = mybir.AluOpType

N_CORES = 8
BC = 128          # images per core
LB = 16           # label images per core (128 / 8)
ITERS = 6         # Hopfield iterations (reference scan converges by iter 2)


# ----------------------------------------------------------------- host prep

def _make_replicas(imgs):
    """[b,1,28,28] -> [128=(j4,xi32), 4*8*b=(phi, yb8, b)], zero-padded 35x32."""
    b = imgs.shape[0]
    pad = np.zeros((b, 35, 32), np.float32)
    pad[:, 2:30, 2:30] = imgs[:, 0]
    out = np.zeros((128, 4 * 8 * b), np.float32)
    for phi in range(4):
        for j in range(4):
            sl = pad[:, phi + j: phi + j + 32: 4, :][:, :8, :]   # [b, 8yb, 32xi]
            out[j * 32:(j + 1) * 32, phi * 8 * b:(phi + 1) * 8 * b] = \
                np.transpose(sl, (2, 1, 0)).reshape(32, 8 * b)
    return out


def _host_prep(inputs):
    """Shared (non-image) constant tensors in device layouts."""
    H = {}
    c1w = np.asarray(inputs['conv1_w'], np.float32)
    c2w = np.asarray(inputs['conv2_w'], np.float32)

    # conv1 Toeplitz weights: [(j,xi),(par,og -> (xq,o8))] packed [128, 896] / [32, 896]
    W1 = np.zeros((2, 4, 128, 112), np.float32)
    W14 = np.zeros((2, 4, 32, 112), np.float32)
    for par in range(2):
        for og in range(4):
            for xq in range(14):
                x = 2 * xq + par
                for dx in range(5):
                    xi = x + dx
                    for j in range(4):
                        W1[par, og, j * 32 + xi, xq * 8:(xq + 1) * 8] = c1w[og * 8:(og + 1) * 8, 0, j, dx]
                    W14[par, og, xi, xq * 8:(xq + 1) * 8] = c1w[og * 8:(og + 1) * 8, 0, 4, dx]
    H['W1SB'] = np.ascontiguousarray(W1.transpose(2, 0, 1, 3).reshape(128, 896))
    H['W14SB'] = np.ascontiguousarray(W14.transpose(2, 0, 1, 3).reshape(32, 896))
    b1 = np.zeros((112, 4), np.float32)
    for og in range(4):
        b1[:, og] = np.tile(np.asarray(inputs['conv1_b'])[og * 8:(og + 1) * 8], 14)
    H['B1SB'] = b1

    # conv2 weights (channel slot = natural channel index og*8+oj)
    c2wp = c2w                                                  # [o64, slot32, 5, 5]
    W2A = np.zeros((5, 128, 128), np.float32)
    W2B = np.zeros((5, 64, 128), np.float32)
    for dy in range(5):
        for j in range(2):
            for xr in range(4):
                dx = xr - j
                if 0 <= dx < 5:
                    W2A[dy, xr * 32:(xr + 1) * 32, j * 64:(j + 1) * 64] = c2wp[:, :, dy, dx].T
            for xr2 in range(2):
                dx = 4 + xr2 - j
                if 0 <= dx < 5:
                    W2B[dy, xr2 * 32:(xr2 + 1) * 32, j * 64:(j + 1) * 64] = c2wp[:, :, dy, dx].T
    H['W2ASB'] = np.ascontiguousarray(W2A.transpose(1, 0, 2).reshape(128, 640))
    H['W2BSB'] = np.ascontiguousarray(W2B.transpose(1, 0, 2).reshape(64, 640))
    H['B2SB'] = np.tile(np.asarray(inputs['conv2_b'], np.float32), 2)[:, None]  # [128,1]

    # fc1 weights: [28 ch=(xh*7+y), 128=(par,o64), 512]
    fw3 = np.asarray(inputs['fc1_w'], np.float32).reshape(512, 64, 7, 7)
    FC1W = np.zeros((28, 128, 512), np.float32)
    for xh in range(4):
        for y in range(7):
            ch = xh * 7 + y
            for par in range(2):
                x = 2 * xh + par
                if x < 7:
                    FC1W[ch, par * 64:(par + 1) * 64, :] = fw3[:, :, y, x].T
    H['FC1W'] = FC1W
    H['FC1B'] = np.ascontiguousarray(np.asarray(inputs['fc1_b'], np.float32).reshape(4, 128).T)

    H['FCNW'] = np.ascontiguousarray(
        np.asarray(inputs['fcn_w'], np.float32).T.reshape(4, 128, 128)
        .transpose(1, 0, 2).reshape(128, 512))                  # [128i, (k,o)]
    H['FCNB'] = np.tile(np.asarray(inputs['fcn_b'], np.float32)[None, :], (128, 1))

    dm = ((1.0 - np.eye(512, dtype=np.float32)) / 128.0).reshape(4, 128, 512)
    H['DMASK'] = np.ascontiguousarray(dm.transpose(1, 0, 2).reshape(128, 2048))
    H['IDENT'] = np.eye(128, dtype=np.float32)
    return H


# ------------------------------------------------------------- device stages

def _conv1_label(nc, tc, c1p, RL, W):
    """fp32 conv1 on the 16-label shard, all 4 phases merged per matmul (N=448).
    Fills c1p [112=(xq,o8), og*14*LB] pooled+relu."""
    b = LB
    R4 = RL[:].rearrange("p (phi yb b) -> p phi yb b", phi=4, yb=8)
    with tc.tile_pool(name="ps1L", bufs=4, space="PSUM") as psum:
        for og in range(4):
            dst = c1p[:, og * 14 * b:(og + 1) * 14 * b].rearrange(
                "p (y w b) -> p y w b", y=7, w=2)
            pp = {}
            for par in range(2):
                ps = psum.tile([112, 4 * 7 * b], F32, tag="p1L", name="p1L")
                lw1 = W['W1SB'][:, (par * 4 + og) * 112:(par * 4 + og + 1) * 112]
                lw4 = W['W14SB'][:, (par * 4 + og) * 112:(par * 4 + og + 1) * 112]
                nc.tensor.matmul(ps[:], lw1, R4[:, :, 0:7, :], start=True, stop=False)
                nc.tensor.matmul(ps[:], lw4, R4[0:32, :, 1:8, :], start=False, stop=True)
                pp[par] = ps[:].rearrange("p (phi y b) -> p phi y b", phi=4, y=7)
            for w2 in range(2):
                d = dst[:, :, w2, :]
                nc.scalar.activation(d, pp[0][:, 2 * w2, :, :], AF.Copy)
                nc.vector.tensor_tensor(d, d, pp[0][:, 2 * w2 + 1, :, :], ALU.max)
                nc.vector.tensor_tensor(d, d, pp[1][:, 2 * w2, :, :], ALU.max)
                nc.vector.tensor_tensor(d, d, pp[1][:, 2 * w2 + 1, :, :], ALU.max)
            sl = c1p[:, og * 14 * b:(og + 1) * 14 * b]
            nc.scalar.activation(sl, sl, AF.Relu, bias=W['B1SB'][:, og:og + 1])


def _conv1_image(nc, tc, c1p, Rsb, W):
    """fp32r conv1, b=128 (per-phase matmuls, N=512/384). Fills c1p."""
    b = BC
    with tc.tile_pool(name="ps1I", bufs=3, space="PSUM") as psum1:
        for og in range(4):
            dst_all = c1p[:, og * 14 * b:(og + 1) * 14 * b].rearrange(
                "p (y w b) -> p y w b", y=7, w=2)
            for phi in range(4):
                pe = psum1.tile([112, 7 * b], F32, tag="p1", name="pe")
                po = psum1.tile([112, 7 * b], F32, tag="p1", name="po")
                for par, ps in ((0, pe), (1, po)):
                    lw1 = W['W1SB'][:, (par * 4 + og) * 112:(par * 4 + og + 1) * 112]
                    lw4 = W['W14SB'][:, (par * 4 + og) * 112:(par * 4 + og + 1) * 112]
                    for lo, hi in ((0, 512), (512, 896)):
                        nc.tensor.matmul(ps[:, lo:hi], lw1,
                                         Rsb[:, phi * 1024 + lo: phi * 1024 + hi],
                                         start=True, stop=False)
                        nc.tensor.matmul(ps[:, lo:hi], lw4,
                                         Rsb[0:32, phi * 1024 + 128 + lo: phi * 1024 + 128 + hi],
                                         start=False, stop=True)
                dst = dst_all[:, :, phi // 2, :]     # even y rows (phi 0,1) / odd (2,3)
                if phi % 2 == 0:
                    nc.scalar.activation(dst, pe[:].rearrange("p (y b) -> p y b", y=7), AF.Copy)
                else:
                    nc.vector.tensor_tensor(dst, dst, pe[:].rearrange("p (y b) -> p y b", y=7), ALU.max)
                nc.vector.tensor_tensor(dst, dst, po[:].rearrange("p (y b) -> p y b", y=7), ALU.max)
            sl = c1p[:, og * 14 * b:(og + 1) * 14 * b]
            nc.scalar.activation(sl, sl, AF.Relu, bias=W['B1SB'][:, og:og + 1])


def _reshuffle(nc, tc, R2, c1p, b, engines):
    """c1p -> conv2 x-replica tiles R2[psi] [128=(xr4,ch32), 4 xb * 18ypad * b].
    DMA dispatch rotated across `engines`."""
    ei = 0
    for xb in range(4):         # xb-major, psi-inner: matches conv2 consumption order
        for psi in (0, 2):
            for xr in range(4):
                xp = psi + 4 * xb + xr - 2
                blk = R2[psi][xr * 32:(xr + 1) * 32, xb * 18 * b:(xb + 1) * 18 * b]
                if not (0 <= xp < 14):
                    nc.gpsimd.memset(blk.bitcast(F32), 0.0)   # never-written slot: zero pad
                    continue
                # zero the y-pad rows (0,1 and 16,17), DMA the 14 data rows
                nc.gpsimd.memset(blk[:, 0:2 * b].bitcast(F32), 0.0)
                nc.gpsimd.memset(blk[:, 16 * b:18 * b].bitcast(F32), 0.0)
                for og in range(4):
                    engines[ei % len(engines)].dma_start(
                        R2[psi][xr * 32 + og * 8: xr * 32 + (og + 1) * 8,
                                xb * 18 * b + 2 * b: xb * 18 * b + 16 * b],
                        c1p[xp * 8:(xp + 1) * 8, og * 14 * b:(og + 1) * 14 * b])
                    ei += 1


def _conv2_label(nc, tc, pooled2, R2, W):
    """fp32 conv2 on label shard, single 14-row y window per xp (N=224)."""
    b = LB
    with tc.tile_pool(name="ps2L", bufs=2, space="PSUM") as psum:
        for xp in range(7):
            psi = (2 * xp) % 4
            xb = (2 * xp - psi) // 4
            par, xh = xp % 2, xp // 2
            ps = psum.tile([128, 14 * b], F32, tag="p2L", name="p2Lps")
            use_b = xp < 6      # xp=6 B-taps read only zero-padded columns
            for dy in range(5):
                base1 = (xb * 18 + dy) * b
                nc.tensor.matmul(ps[:],
                                 W['W2ASB'][:, dy * 128:(dy + 1) * 128],
                                 R2[psi][:, base1: base1 + 14 * b],
                                 start=(dy == 0), stop=(dy == 4 and not use_b))
                if use_b:
                    base2 = ((xb + 1) * 18 + dy) * b
                    nc.tensor.matmul(ps[:],
                                     W['W2BSB'][:, dy * 128:(dy + 1) * 128],
                                     R2[psi][0:64, base2: base2 + 14 * b],
                                     start=False, stop=(dy == 4))
            pv = ps[:].rearrange("p (r w b) -> p r w b", r=7, w=2)
            dst = pooled2[par * 64:(par + 1) * 64, xh * 7 * b:(xh + 1) * 7 * b] \
                .rearrange("p (r b) -> p r b", r=7)
            nc.scalar.activation(dst, pv[0:64, :, 0, :], AF.Copy)
            nc.vector.tensor_tensor(dst, dst, pv[0:64, :, 1, :], ALU.max)
            nc.vector.tensor_tensor(dst, dst, pv[64:128, :, 0, :], ALU.max)
            nc.vector.tensor_tensor(dst, dst, pv[64:128, :, 1, :], ALU.max)
    nc.vector.memset(pooled2[64:128, 3 * 7 * b:4 * 7 * b], 0.0)
    nc.scalar.activation(pooled2[:], pooled2[:], AF.Relu, bias=W['B2SB'][:, 0:1])


def _conv2_image(nc, tc, pooled2, R2, W):
    """fp32r conv2, b=128 (two y windows per xp)."""
    b = BC
    with tc.tile_pool(name="ps2I", bufs=2, space="PSUM") as psum2:
        for xp in range(7):
            psi = (2 * xp) % 4
            xb = (2 * xp - psi) // 4
            par, xh = xp % 2, xp // 2
            use_b = xp < 6      # xp=6 B-taps read only zero-padded columns
            for (y0, ny) in ((0, 8), (8, 6)):
                nylen = ny * b
                ps = psum2.tile([128, 8 * b], F32, tag="p2", name="p2ps")
                splits = [(0, 512), (512, nylen)]
                for (lo, hi) in splits:
                    first = True
                    for dy in range(5):
                        base1 = (xb * 18 + y0 + dy) * b
                        nc.tensor.matmul(ps[:, lo:hi],
                                         W['W2ASB'][:, dy * 128:(dy + 1) * 128],
                                         R2[psi][:, base1 + lo: base1 + hi],
                                         start=first, stop=(dy == 4 and not use_b))
                        first = False
                        if use_b:
                            base2 = ((xb + 1) * 18 + y0 + dy) * b
                            nc.tensor.matmul(ps[:, lo:hi],
                                             W['W2BSB'][:, dy * 128:(dy + 1) * 128],
                                             R2[psi][0:64, base2 + lo: base2 + hi],
                                             start=False, stop=(dy == 4))
                nr = ny // 2
                pv = ps[:, 0:nylen].rearrange("p (r w b) -> p r w b", r=nr, w=2)
                dst = pooled2[par * 64:(par + 1) * 64,
                              xh * 7 * b + (y0 // 2) * b: xh * 7 * b + (y0 // 2 + nr) * b] \
                    .rearrange("p (r b) -> p r b", r=nr)
                nc.scalar.activation(dst, pv[0:64, :, 0, :], AF.Copy)
                nc.vector.tensor_tensor(dst, dst, pv[0:64, :, 1, :], ALU.max)
                nc.vector.tensor_tensor(dst, dst, pv[64:128, :, 0, :], ALU.max)
                nc.vector.tensor_tensor(dst, dst, pv[64:128, :, 1, :], ALU.max)
    nc.gpsimd.memset(pooled2[64:128, 3 * 7 * b:4 * 7 * b].bitcast(F32), 0.0)
    nc.scalar.activation(pooled2[:], pooled2[:], AF.Relu, bias=W['B2SB'][:, 0:1])


def build_program():
    """Build the full Bass program; returns (nc, input_names, output_names)."""
    nc = bacc.Bacc("TRN2", target_bir_lowering=False, debug=False, num_devices=N_CORES)
    b = BC

    din = {}
    RDT = {'R1'}   # only the image replicas may be rounded at input staging

    def dram_in(name, shape):
        dt = F32R if name in RDT else F32
        din[name] = nc.dram_tensor(name, list(shape), dt, kind="ExternalInput").ap()

    for name, shape in [('R1', (128, 4096)), ('R1L', (128, 4 * 8 * LB)),
                        ('W1SB', (128, 896)), ('W14SB', (32, 896)), ('B1SB', (112, 4)),
                        ('W2ASB', (128, 640)), ('W2BSB', (64, 640)), ('B2SB', (128, 1)),
                        ('FC1W', (28, 128, 512)), ('FC1B', (128, 4)),
                        ('FCNW', (128, 512)), ('FCNB', (128, 128)),
                        ('DMASK', (128, 2048)), ('IDENT', (128, 128))]:
        dram_in(name, shape)
    out_d = nc.dram_tensor('OUT', [128, 128], F32, kind="ExternalOutput").ap()
    lbl_d = nc.dram_tensor('LABEL', [128, 128], F32, kind="ExternalOutput").ap()
    dbg_d = nc.dram_tensor('DBGREP', [128, 512], F32, kind="ExternalOutput").ap()
    dbl_d = nc.dram_tensor('DBGLAT', [128, 512], F32, kind="ExternalOutput").ap()
    lpiece = nc.dram_tensor('LPIECE', [4, 128, LB], F32, kind="Internal").ap()
    lgath = nc.dram_tensor('LGATH', [N_CORES, 4, 128, LB], F32, kind="Internal",
                           addr_space="Shared").ap()

    with tile.TileContext(nc) as tc, contextlib.ExitStack() as ctx:
        wpool = ctx.enter_context(tc.tile_pool(name="weights", bufs=1))
        cpool = ctx.enter_context(tc.tile_pool(name="persist", bufs=1))

        W = {}
        for name, shape in [('W1SB', (128, 896)), ('W14SB', (32, 896)), ('B1SB', (112, 4)),
                            ('W2ASB', (128, 640)), ('W2BSB', (64, 640)), ('B2SB', (128, 1)),
                            ('FC1B', (128, 4)), ('FCNW', (128, 512)), ('FCNB', (128, 128)),
                            ('DMASK', (128, 2048)), ('IDENT', (128, 128))]:
            t = wpool.tile(list(shape), F32R if name in RDT else F32, tag=name, name=name)
            nc.sync.dma_start(t[:], din[name][:])
            W[name] = t
        ones_col = wpool.tile([128, 1], F32, tag="ones_col", name="ones_col")
        nc.vector.memset(ones_col[:], 1.0)
        ones_row = wpool.tile([1, 128], F32, tag="ones_row", name="ones_row")
        nc.vector.memset(ones_row[:], 1.0)

        # fp32r copies of the conv weights for the image branch (the exact F32
        # tiles feed the label branch; writing an F32R tile rounds to 12-bit)
        WR = {}
        for name, shape in [('W1SB', (128, 896)), ('W14SB', (32, 896)),
                            ('W2ASB', (128, 640)), ('W2BSB', (64, 640))]:
            t = wpool.tile(list(shape), F32R, tag=name + 'R', name=name + 'R')
            nc.scalar.activation(t[:], W[name][:], AF.Copy)
            WR[name] = t
        for name in ('B1SB', 'B2SB', 'FC1B', 'IDENT'):
            WR[name] = W[name]

        repT = [cpool.tile([128, 128], F32, tag=f"repT{k}", name=f"repT{k}")
                for k in range(4)]
        latT = [cpool.tile([128, 128], F32, tag=f"latT{k}", name=f"latT{k}")
                for k in range(4)]

        # Pool stack, opened bottom-up in reverse death order (LIFO allocator):
        # pooled2I > R2I > pooled2L > R2L > (RI,c1pI) > (RL,c1pL)
        s_p2I = contextlib.ExitStack()
        pooled2I = s_p2I.enter_context(tc.tile_pool(name="p2I", bufs=1)) \
            .tile([128, 4 * 7 * BC], F32R, name="pooled2I")
        s_R2I = contextlib.ExitStack()
        R2I = {psi: s_R2I.enter_context(tc.tile_pool(name=f"r2_{psi}I", bufs=1))
               .tile([128, 4 * 18 * BC], F32R, name=f"r2_{psi}I") for psi in (0, 2)}
        s_p2L = contextlib.ExitStack()
        pooled2L = s_p2L.enter_context(tc.tile_pool(name="p2L", bufs=1)) \
            .tile([128, 4 * 7 * LB], F32, name="pooled2L")
        s_R2L = contextlib.ExitStack()
        R2L = {psi: s_R2L.enter_context(tc.tile_pool(name=f"r2_{psi}L", bufs=1))
               .tile([128, 4 * 18 * LB], F32, name=f"r2_{psi}L") for psi in (0, 2)}
        s_img1 = contextlib.ExitStack()
        RI = s_img1.enter_context(tc.tile_pool(name="repl_I", bufs=1)) \
            .tile([128, 4096], F32R, name="RI")
        c1pI = s_img1.enter_context(tc.tile_pool(name="c1pI", bufs=1)) \
            .tile([112, 4 * 14 * BC], F32R, name="c1pI")
        s_lbl1 = contextlib.ExitStack()
        RL = s_lbl1.enter_context(tc.tile_pool(name="repl_L", bufs=1)) \
            .tile([128, 4 * 8 * LB], F32, name="RL")
        c1pL = s_lbl1.enter_context(tc.tile_pool(name="c1pL", bufs=1)) \
            .tile([112, 4 * 14 * LB], F32, name="c1pL")

        # ---- label conv1 (fp32, b=16) ----
        nc.sync.dma_start(RL[:], din['R1L'][:])
        _conv1_label(nc, tc, c1pL, RL, W)

        # ---- label reshuffle (gpsimd queue only, runs during image conv1) ----
        _reshuffle(nc, tc, R2L, c1pL, LB, [nc.gpsimd])
        s_lbl1.close()                      # RL, c1pL dead

        # ---- image conv1 (fp32r, b=128) ----
        for phi in range(4):
            nc.sync.dma_start(RI[:, phi * 1024:(phi + 1) * 1024],
                              din['R1'][:, phi * 1024:(phi + 1) * 1024])
        _conv1_image(nc, tc, c1pI, RI, WR)

        # ---- image reshuffle (sync/gpsimd queues, during label conv2/fc1) ----
        _reshuffle(nc, tc, R2I, c1pI, BC, [nc.sync, nc.gpsimd])
        s_img1.close()                      # RI, c1pI dead

        # ---- label conv2 + fc1 + AllGather ----
        _conv2_label(nc, tc, pooled2L, R2L, W)
        s_R2L.close()                       # R2L dead
        with tc.tile_pool(name="fc1wL", bufs=4) as fc1wp, \
             tc.tile_pool(name="ps3L", bufs=1, space="PSUM") as psum3, \
             tc.tile_pool(name="repTL", bufs=1) as rtlp:
            latL_ps = [psum3.tile([128, LB], F32, tag=f"latL{lt}", name=f"latL{lt}")
                       for lt in range(4)]
            for ch in range(28):
                wt = fc1wp.tile([128, 512], F32, tag="fc1wL", name="fc1wL")
                nc.scalar.dma_start(wt[:], din['FC1W'][ch, :, :])
                for lt in range(4):
                    nc.tensor.matmul(latL_ps[lt][:],
                                     wt[:, lt * 128:(lt + 1) * 128],
                                     pooled2L[:, ch * LB:(ch + 1) * LB],
                                     start=(ch == 0), stop=(ch == 27))
            for lt in range(4):
                rT = rtlp.tile([128, LB], F32, tag=f"repTL{lt}", name=f"repTL{lt}")
                nc.scalar.activation(rT[:], latL_ps[lt][:], AF.Tanh,
                                     bias=W['FC1B'][:, lt:lt + 1])
                nc.sync.dma_start(lpiece[lt], rT[:])
            nc.gpsimd.collective_compute(
                "AllGather", mybir.AluOpType.bypass,
                replica_groups=[list(range(N_CORES))],
                ins=[lpiece[:]], outs=[lgath[:]])
            for cc in range(N_CORES):
                for lt in range(4):
                    nc.gpsimd.dma_start(repT[lt][:, cc * LB:(cc + 1) * LB],
                                        lgath[cc, lt])
        s_p2L.close()                       # pooled2L dead
        for lt in range(4):
            nc.sync.dma_start(dbg_d[:, lt * 128:(lt + 1) * 128], repT[lt][:])

        # ---- image conv2 (fp32r) ----
        _conv2_image(nc, tc, pooled2I, R2I, WR)
        s_R2I.close()                       # R2I dead

        # ---- hopfield w (fp32, needs gathered repT) ----
        w_sb = cpool.tile([128, 2048], F32, tag="w", name="w_sb")
        with tc.tile_pool(name="wb_sb", bufs=1) as sp, \
             tc.tile_pool(name="wb_ps", bufs=1, space="PSUM") as pp:
            parts = sp.tile([128, 4], F32, name="parts")
            for k in range(4):
                nc.vector.tensor_reduce(parts[:, k:k + 1], repT[k][:],
                                        mybir.AxisListType.X, ALU.add)
            rsum = sp.tile([128, 1], F32, name="rsum")
            nc.vector.tensor_tensor(rsum[:], parts[:, 0:1], parts[:, 1:2], ALU.add)
            nc.vector.tensor_tensor(rsum[:], rsum[:], parts[:, 2:3], ALU.add)
            nc.vector.tensor_tensor(rsum[:], rsum[:], parts[:, 3:4], ALU.add)
            tot_ps = pp.tile([1, 1], F32, tag="tot", name="tot_ps")
            nc.tensor.matmul(tot_ps[:], rsum[:], ones_col[:], start=True, stop=True)
            rho1 = sp.tile([1, 1], F32, name="rho1")
            nc.scalar.activation(rho1[:], tot_ps[:], AF.Copy, scale=1.0 / 65536.0)
            rho_ps = pp.tile([128, 1], F32, tag="rhob", name="rho_ps")
            nc.tensor.matmul(rho_ps[:], ones_row[:], rho1[:], start=True, stop=True)
            rho_col = sp.tile([128, 1], F32, name="rho_col")
            nc.scalar.activation(rho_col[:], rho_ps[:], AF.Copy)
            tB = sp.tile([128, 512], F32, name="tB")
            tb_ps = pp.tile([128, 512], F32, tag="tbps", name="tb_ps")
            for k in range(4):
                tT = sp.tile([128, 128], F32, tag="tT", name="tT", bufs=2)
                nc.vector.tensor_scalar(tT[:], repT[k][:], rho_col[:], None, ALU.subtract)
                nc.tensor.transpose(tb_ps[:, k * 128:(k + 1) * 128], tT[:], W['IDENT'][:])
            nc.scalar.activation(tB[:], tb_ps[:], AF.Copy)
            for jc in range(4):
                w_ps = pp.tile([128, 512], F32, tag="wps", name="w_ps", bufs=2)
                nc.tensor.matmul(w_ps[:], tB[:, jc * 128:(jc + 1) * 128], tB[:],
                                 start=True, stop=True)
                nc.vector.tensor_tensor(w_sb[:, jc * 512:(jc + 1) * 512], w_ps[:],
                                        W['DMASK'][:, jc * 512:(jc + 1) * 512], ALU.mult)

        # ---- image fc1 (fp32r batch-major) + transpose to latent-major ----
        with tc.tile_pool(name="fc1wI", bufs=4) as fc1wp, \
             tc.tile_pool(name="ps3I", bufs=1, space="PSUM") as psum3, \
             tc.tile_pool(name="latbm", bufs=1) as lbp:
            lat_ps = psum3.tile([128, 512], F32, tag="latbm", name="lat_ps")
            for ch in range(28):
                wt = fc1wp.tile([128, 512], F32, tag="fc1wI", name="fc1wI")
                nc.sync.dma_start(wt[:], din['FC1W'][ch, :, :])
                wtr = fc1wp.tile([128, 512], F32R, tag="fc1wIr", name="fc1wIr")
                nc.scalar.activation(wtr[:], wt[:], AF.Copy)
                nc.tensor.matmul(lat_ps[:], pooled2I[:, ch * b:(ch + 1) * b], wtr[:],
                                 start=(ch == 0), stop=(ch == 27))
            lat_sb = lbp.tile([128, 512], F32, name="lat_sb")
            nc.scalar.activation(lat_sb[:], lat_ps[:], AF.Copy)
            with tc.tile_pool(name="trps", bufs=2, space="PSUM") as trp:
                for lt in range(4):
                    tp = trp.tile([128, 128], F32, tag="tr", name="tr")
                    nc.tensor.transpose(tp[:], lat_sb[:, lt * 128:(lt + 1) * 128],
                                        W['IDENT'][:])
                    nc.scalar.activation(latT[lt][:], tp[:], AF.Identity,
                                         bias=W['FC1B'][:, lt:lt + 1])
        s_p2I.close()                       # pooled2I dead
        for lt in range(4):
            nc.sync.dma_start(dbl_d[:, lt * 128:(lt + 1) * 128], latT[lt][:])

        # ---- clustering + heads ----
        with tc.tile_pool(name="clv", bufs=2) as vpool, \
             tc.tile_pool(name="cl_ps", bufs=1, space="PSUM") as cps:
            s_mag, scur = [], []
            for k in range(4):
                s0 = cpool.tile([128, b], F32, tag=f"s0_{k}", name=f"s0_{k}")
                nc.scalar.activation(s0[:], latT[k][:], AF.Tanh)
                sm = cpool.tile([128, b], F32, tag=f"smag{k}", name=f"smag{k}")
                nc.scalar.activation(sm[:], s0[:], AF.Abs)
                s_mag.append(sm)
                scur.append(s0)
            min_e = cpool.tile([1, b], F32, tag="min_e", name="min_e")
            nc.vector.memset(min_e[:], 3.0e38)   # +inf stand-in (sim finite-check)
            min_s = []
            for k in range(4):
                t = cpool.tile([128, b], F32, tag=f"mins{k}", name=f"mins{k}")
                nc.vector.memset(t[:], 0.0)
                min_s.append(t)

            def mm_h(src):
                ps = cps.tile([128, 512], F32, tag="h", name="h_ps", bufs=2)
                for i in range(4):
                    for jc in range(4):
                        nc.tensor.matmul(ps[:, i * 128:(i + 1) * 128],
                                         w_sb[:, jc * 512 + i * 128: jc * 512 + (i + 1) * 128],
                                         src[jc][:], start=(jc == 0), stop=(jc == 3))
                return ps

            h = mm_h(scur)
            for it in range(ITERS):
                snew = []
                for i in range(4):
                    sg = vpool.tile([128, b], F32, tag="sg", name="sg")
                    nc.scalar.activation(sg[:], h[:, i * 128:(i + 1) * 128], AF.Sign)
                    sn = vpool.tile([128, b], F32, tag=f"sn{i}", name=f"sn{i}")
                    nc.vector.tensor_tensor(sn[:], s_mag[i][:], sg[:], ALU.mult)
                    snew.append(sn)
                h = mm_h(snew)
                e_ps = cps.tile([1, b], F32, tag="e", name="e_ps", bufs=2)
                for i in range(4):
                    pr = vpool.tile([128, b], F32, tag="pr", name="pr")
                    nc.vector.tensor_tensor(pr[:], snew[i][:], h[:, i * 128:(i + 1) * 128], ALU.mult)
                    nc.tensor.matmul(e_ps[:], ones_col[:], pr[:], start=(i == 0), stop=(i == 3))
                e_row = vpool.tile([1, b], F32, tag="erow", name="e_row")
                nc.vector.tensor_scalar(e_row[:], e_ps[:], -1.0, None, ALU.mult)
                mask = vpool.tile([1, b], F32, tag="mask", name="mask")
                nc.vector.tensor_tensor(mask[:], e_row[:], min_e[:], ALU.is_lt)
                mask_i = vpool.tile([1, b], mybir.dt.int32, tag="mask_i", name="mask_i")
                nc.vector.tensor_copy(mask_i[:], mask[:])
                nc.vector.copy_predicated(min_e[:], mask_i[:], e_row[:])
                mb_ps = cps.tile([128, b], F32, tag="mb", name="mb_ps", bufs=2)
                nc.tensor.matmul(mb_ps[:], ones_row[:], mask[:], start=True, stop=True)
                mb_i = vpool.tile([128, b], mybir.dt.int32, tag="mb_i", name="mb_i")
                nc.vector.tensor_copy(mb_i[:], mb_ps[:])
                for i in range(4):
                    nc.vector.copy_predicated(min_s[i][:], mb_i[:], snew[i][:])
                scur = snew

            # ---- heads ----
            for head in ('out', 'label'):
                lg_ps = cps.tile([128, 128], F32, tag=f"lg_{head}", name=f"lg_{head}")
                if head == 'out':
                    for k in range(4):
                        nc.tensor.matmul(lg_ps[:], min_s[k][:], repT[k][:],
                                         start=(k == 0), stop=(k == 3))
                    logits = vpool.tile([128, 128], F32, tag="lgs", name="lgs")
                    nc.scalar.activation(logits[:], lg_ps[:], AF.Abs)
                else:
                    for k in range(4):
                        nc.tensor.matmul(lg_ps[:], latT[k][:],
                                         W['FCNW'][:, k * 128:(k + 1) * 128],
                                         start=(k == 0), stop=(k == 3))
                    logits = vpool.tile([128, 128], F32, tag="lgs2", name="lgs2")
                    nc.vector.tensor_tensor(logits[:], lg_ps[:], W['FCNB'][:], ALU.add)
                mx = vpool.tile([128, 1], F32, tag="mx", name="mx")
                nc.vector.tensor_reduce(mx[:], logits[:], mybir.AxisListType.X, ALU.max)
                mxn = vpool.tile([128, 1], F32, tag="mxn", name="mxn")
                nc.vector.tensor_scalar(mxn[:], mx[:], -1.0, None, ALU.mult)
                ex = vpool.tile([128, 128], F32, tag="ex", name="ex")
                nc.scalar.activation(ex[:], logits[:], AF.Exp, bias=mxn[:])
                sme = vpool.tile([128, 1], F32, tag="sme", name="sme")
                nc.vector.tensor_reduce(sme[:], ex[:], mybir.AxisListType.X, ALU.add)
                rec = vpool.tile([128, 1], F32, tag="rec", name="rec")
                nc.vector.reciprocal(rec[:], sme[:])
                prob = vpool.tile([128, 128], F32, tag="prob", name="prob")
                nc.vector.tensor_scalar(prob[:], ex[:], rec[:], None, ALU.mult)
                nc.sync.dma_start((out_d if head == 'out' else lbl_d)[:], prob[:])

    nc.compile()
    in_names = list(din.keys())
    return nc, in_names, ['OUT', 'LABEL']


# --------------------------------------------------------------- entry point

_CACHE = {}
TRACE = False     # set True (e.g. from test.py) to capture a neuron profile


def kernel(**inputs):
    if 'prog' not in _CACHE:
        _CACHE['prog'] = build_program()
    nc, in_names, out_names = _CACHE['prog']

    H = _host_prep(inputs)
    image = np.asarray(inputs['image'], np.float32)
    label_images = np.asarray(inputs['label_images'], np.float32)
    shared = {k: H[k] for k in ['W1SB', 'W14SB', 'B1SB', 'W2ASB', 'W2BSB', 'B2SB',
                                'FC1W', 'FC1B', 'FCNW', 'FCNB', 'DMASK', 'IDENT']}
    in_maps = []
    for c in range(N_CORES):
        m = dict(shared)
        m['R1'] = _make_replicas(image[c * BC:(c + 1) * BC])
        m['R1L'] = _make_replicas(label_images[c * LB:(c + 1) * LB])
        in_maps.append(m)

    res = bass_utils.run_bass_kernel_spmd(nc, in_maps, core_ids=list(range(N_CORES)),
                                          trace=TRACE)
    _CACHE['last_results'] = res
    outs = np.concatenate([res.results[c]['OUT'] for c in range(N_CORES)], axis=0)
    labels = np.concatenate([res.results[c]['LABEL'] for c in range(N_CORES)], axis=0)
    return outs, labels
